# revision 25
# baseline (speedup 1.0000x reference)
"""MultiHeadAttention forward on 8 trn2 NeuronCores.

Sharding: 32 (batch, head) pairs across 8 cores -> 4 heads of one batch per
core (cores 0-3: batch 0, cores 4-7: batch 1).  Each core computes Q/K/V
projections for its head slice, scores softmax and attn@V, plus a partial
output projection; the host sums the 4 partials per batch (replacing the
all-reduce) and adds the bias.

On-device everything is kept feature-on-partitions ("transposed"):
  Q^T/K^T: [dk, q] ; V: [k, dv] ; scores^T strips: [k=128, q] ;
  E^T = exp(s^T/8) strips stay in SBUF; attn@V is computed as
  [V | ones]^T-augmented matmuls so softmax row sums come out of the same
  accumulation (row 64 of the PSUM tile).  Normalized attn^T is DMA'd out
  per head as [k, q]; the host transposes back to [q, k] when assembling
  the full [B, H, S, S] output.
"""

import os
import numpy as np
from concurrent.futures import ThreadPoolExecutor
from contextlib import ExitStack

os.environ.setdefault("MYCRO_LOCAL_CACHE", "1")

B, S, D, H, DK = 2, 2048, 1024, 16, 64
HPC = H // 4  # heads per core (4)
NCORES = 8
SCALE = 0.125  # 1 / sqrt(DK)

# --- configuration -------------------------------------------------------
# "f32":  E^T strips fp32, attn output fp32, q-chunk 1024 (SBUF capacity)
# "bf16": E^T strips bf16, attn output bf16 (host upcasts), q-chunk 2048
MODE = os.environ.get("MHA_MODE", "f32")
ET_BUFS = int(os.environ.get("MHA_ET_BUFS", "17"))

_cache = {}


def _build(mode):
    import concourse.bass as bass
    import concourse.mybir as mybir
    import concourse.tile as tile

    f32 = mybir.dt.float32
    f32r = mybir.dt.float32r
    bf16 = mybir.dt.bfloat16

    if mode == "bf16":
        # everything bf16 except PSUM accumulation, r, and out_p partials
        C_DT, E_DT, QCH = bf16, bf16, 1024
        ATTN_DT = f32 if os.environ.get("MHA_ATTN_F32") == "1" else bf16
        et_bufs = int(os.environ.get("MHA_ET_BUFS", "34"))
        pev_bufs = 2
    else:
        C_DT, E_DT, ATTN_DT, QCH = f32r, f32r, f32, 1024
        et_bufs = int(os.environ.get("MHA_ET_BUFS", "17"))
        pev_bufs = 1
    NQC = S // QCH

    nc = bass.Bass()

    xq_t = nc.dram_tensor("xq_t", [D, S], C_DT, kind="ExternalInput")
    xk_t = nc.dram_tensor("xk_t", [D, S], C_DT, kind="ExternalInput")
    xv_t = nc.dram_tensor("xv_t", [D, S], C_DT, kind="ExternalInput")
    wq_t = nc.dram_tensor("wq_t", [D, HPC * DK], C_DT, kind="ExternalInput")
    wk_t = nc.dram_tensor("wk_t", [D, HPC * DK], C_DT, kind="ExternalInput")
    wv_t = nc.dram_tensor("wv_t", [D, HPC * DK], C_DT, kind="ExternalInput")
    wo_t = nc.dram_tensor("wo_t", [HPC * DK, D], C_DT, kind="ExternalInput")
    attn_t = nc.dram_tensor("attn_t", [HPC, S, S], ATTN_DT, kind="ExternalOutput")
    out_p = nc.dram_tensor("out_p", [S, D], f32, kind="ExternalOutput")

    Exp = mybir.ActivationFunctionType.Exp

    with ExitStack() as ctx:
        tc = ctx.enter_context(tile.TileContext(nc))
        wpool = ctx.enter_context(tc.tile_pool(name="wpool", bufs=1))
        big = ctx.enter_context(tc.tile_pool(name="big", bufs=1))
        xstream = ctx.enter_context(tc.tile_pool(name="xstream", bufs=4))
        et_pool = ctx.enter_context(tc.tile_pool(name="et_pool", bufs=et_bufs))
        stage = ctx.enter_context(tc.tile_pool(name="stage", bufs=3))
        rpool = ctx.enter_context(tc.tile_pool(name="rpool", bufs=2))
        dpool = ctx.enter_context(tc.tile_pool(name="dpool", bufs=2, space="DRAM"))
        ps = ctx.enter_context(tc.tile_pool(name="ps", bufs=2, space="PSUM"))
        pev_pool = ctx.enter_context(tc.tile_pool(name="pev_pool", bufs=pev_bufs, space="PSUM"))

        # ---- resident tensors ----
        q_t0 = big.tile([128, S], C_DT)  # Q^T heads 0,1  [dk(2 heads), q]
        q_t1 = big.tile([128, S], C_DT)  # Q^T heads 2,3
        k_t0 = big.tile([128, S], C_DT)
        k_t1 = big.tile([128, S], C_DT)
        v_sb = big.tile([128, 16, HPC, 65], E_DT)  # [k%128, kstrip, h, dv|1]
        out_avT = big.tile([64, HPC, S], C_DT)  # attn@V result, [dv, h, q]
        wo_sb = big.tile([64, HPC, D], C_DT)  # w_o^T as [dv, h, n]

        # weights for projections — one shared slot, loaded per phase
        wq_sb = wpool.tile([128, 8, HPC * DK], C_DT, tag="w")
        wk_sb = wpool.tile([128, 8, HPC * DK], C_DT, tag="w")
        wv_sb = wpool.tile([128, 8, HPC * DK], C_DT, tag="w")

        nc.sync.dma_start(out=wq_sb, in_=wq_t[:].rearrange("(c p) d -> p c d", p=128))
        nc.sync.dma_start(out=wk_sb, in_=wk_t[:].rearrange("(c p) d -> p c d", p=128))
        nc.sync.dma_start(out=wv_sb, in_=wv_t[:].rearrange("(c p) d -> p c d", p=128))
        nc.sync.dma_start(out=wo_sb, in_=wo_t[:].rearrange("(h p) n -> p h n", p=64))
        # col 64 stays 1.0 (ones column -> softmax row sums); f32r needs a
        # plain-f32 bitcast view for memset
        nc.vector.memset(v_sb.bitcast(f32) if E_DT == f32r else v_sb, 1.0)

        # ---- phase P: projections ----
        # Q^T and K^T: [dk-group(128) x q] accumulated over 8 dm-chunks
        for name, x_dram, w_sb, dst in (
            ("q", xq_t, wq_sb, (q_t0, q_t1)),
            ("k", xk_t, wk_sb, (k_t0, k_t1)),
        ):
            for qh in range(2):  # q halves of 1024
                pg0 = ps.tile([128, 1024], f32, name=f"pg0_{name}_{qh}", tag="ps")
                pg1 = ps.tile([128, 1024], f32, name=f"pg1_{name}_{qh}", tag="ps")
                pgs = (pg0, pg1)
                for c in range(8):
                    xch = xstream.tile([128, 1024], C_DT, name=f"xch_{name}_{qh}_{c}", tag="xs")
                    nc.sync.dma_start(
                        out=xch,
                        in_=x_dram[c * 128:(c + 1) * 128, qh * 1024:(qh + 1) * 1024],
                    )
                    for g in range(2):
                        for jj in range(2):
                            nc.tensor.matmul(
                                pgs[g][:, jj * 512:(jj + 1) * 512],
                                w_sb[:, c, g * 128:(g + 1) * 128],
                                xch[:, jj * 512:(jj + 1) * 512],
                                start=(c == 0),
                                stop=(c == 7),
                            )
                for g in range(2):
                    nc.vector.tensor_copy(
                        out=dst[g][:, qh * 1024:(qh + 1) * 1024], in_=pgs[g]
                    )

        # V: [k x dv] accumulated over 8 dm-chunks, per k-strip
        for i in range(16):
            xvc = xstream.tile([128, 8, 128], C_DT, name=f"xvc_{i}", tag="xs")
            nc.sync.dma_start(
                out=xvc,
                in_=xv_t[:].rearrange("(c p) s -> p c s", p=128)[:, :, i * 128:(i + 1) * 128],
            )
            pv = ps.tile([128, HPC * DK], f32, name=f"pv_{i}", tag="ps")
            for c in range(8):
                nc.tensor.matmul(
                    pv,
                    xvc[:, c, :],
                    wv_sb[:, c, :],
                    start=(c == 0),
                    stop=(c == 7),
                )
            nc.vector.tensor_copy(out=v_sb[:, i, :, 0:64], in_=pv[:].rearrange("p (h d) -> p h d", h=HPC))

        # ---- phase A: attention, q-chunk outer so the output projection
        # for chunk qc overlaps the attention of chunk qc+1 ----
        for qc in range(NQC):
            for h in range(HPC):
                q_t = (q_t0, q_t1)[h // 2]
                k_t = (k_t0, k_t1)[h // 2]
                off = (h % 2) * 64
                q0 = qc * QCH
                ets = []
                for i in range(16):
                    et = et_pool.tile([128, QCH], E_DT, name=f"et_{h}_{qc}_{i}", tag="et")
                    ets.append(et)
                    for qq in range(QCH // 1024):
                        pss = ps.tile([128, 1024], f32, name=f"pss_{h}_{qc}_{i}_{qq}", tag="ps")
                        for j2 in range(2):
                            qlo = q0 + qq * 1024 + j2 * 512
                            nc.tensor.matmul(
                                pss[:, j2 * 512:(j2 + 1) * 512],
                                k_t[off:off + 64, i * 128:(i + 1) * 128],
                                q_t[off:off + 64, qlo:qlo + 512],
                                start=True,
                                stop=True,
                            )
                        nc.scalar.activation(
                            out=et[:, qq * 1024:(qq + 1) * 1024],
                            in_=pss,
                            func=Exp,
                            scale=SCALE,
                        )
                # attn @ V with ones-augmented V: row 64 = softmax sums.
                # i-outer: EV matmuls for strip i become ready as soon as
                # exp(i) lands, so the PE can fill exp-wait gaps.
                pev = pev_pool.tile([65, QCH], f32, name=f"pev_{h}_{qc}", tag="pev")
                for i in range(16):
                    for j in range(QCH // 512):
                        nc.tensor.matmul(
                            pev[:, j * 512:(j + 1) * 512],
                            v_sb[:, i, h, :],
                            ets[i][:, j * 512:(j + 1) * 512],
                            start=(i == 0),
                            stop=(i == 15),
                        )
                # Softmax denominators: DVE reciprocal costs ~6.5ns per
                # FREE element regardless of partition count, so bounce the
                # [1,QCH] rowsums through DRAM into a [128, QCH/128] layout,
                # reciprocal there (free dim = QCH/128), bounce back, and
                # broadcast-load to all partitions.
                r_sb = rpool.tile([1, QCH], f32, name=f"r_sb_{h}_{qc}", tag="r", bufs=2)
                nc.vector.tensor_copy(out=r_sb, in_=pev[64:65, :])
                r_dram = dpool.tile([1, QCH], f32, name=f"r_dram_{h}_{qc}", tag="rd")
                nc.sync.dma_start(out=r_dram, in_=r_sb)
                rs_sm = rpool.tile([128, QCH // 128], f32, name=f"rs_sm_{h}_{qc}", tag="rs", bufs=2)
                nc.sync.dma_start(
                    out=rs_sm, in_=r_dram[0].rearrange("(p c) -> p c", p=128))
                nc.vector.reciprocal(out=rs_sm, in_=rs_sm)
                r2_dram = dpool.tile([128, QCH // 128], f32, name=f"r2_dram_{h}_{qc}", tag="rd2")
                nc.sync.dma_start(out=r2_dram, in_=rs_sm)
                rb = rpool.tile([128, QCH], f32, name=f"rb_{h}_{qc}", tag="rb")
                nc.sync.dma_start(
                    out=rb,
                    in_=r2_dram[:].rearrange("p c -> (p c)")[None, :].to_broadcast([128, QCH]))
                # normalized attn@V -> out_avT
                nc.vector.tensor_mul(
                    out_avT[:, h, q0:q0 + QCH], pev[0:64, :], rb[0:64, :]
                )
                # normalized attn strips -> HBM (transposed layout [k, q])
                for i in range(16):
                    ast = stage.tile([128, QCH], ATTN_DT, name=f"ast_{h}_{qc}_{i}", tag="ast")
                    nc.vector.tensor_mul(ast, ets[i], rb)
                    nc.sync.dma_start(
                        out=attn_t[h, i * 128:(i + 1) * 128, q0:q0 + QCH], in_=ast
                    )

            # ---- output projection partial for this q-chunk ----
            for qs in range(qc * (QCH // 128), (qc + 1) * (QCH // 128)):
                po = ps.tile([128, D], f32, name=f"po_{qs}", tag="ps")
                for n2 in range(2):
                    for h in range(HPC):
                        nc.tensor.matmul(
                            po[:, n2 * 512:(n2 + 1) * 512],
                            out_avT[:, h, qs * 128:(qs + 1) * 128],
                            wo_sb[:, h, n2 * 512:(n2 + 1) * 512],
                            start=(h == 0),
                            stop=(h == 3),
                        )
                ost = stage.tile([128, D], f32, name=f"ost_{qs}", tag="ast")
                nc.vector.tensor_copy(out=ost, in_=po)
                nc.sync.dma_start(out=out_p[qs * 128:(qs + 1) * 128, :], in_=ost)

    _split_waits(nc, mybir)
    return nc


def _split_waits(nc, mybir, mm_limit=1, other_limit=1):
    """The walrus build in this env accepts only a small number of sync-wait
    commands per instruction (matmul LDWEIGHTS appears to take just one).
    Hoist excess waits onto injected same-engine NoOps, which execute the
    waits in order before the real instruction."""
    nid = [0]

    def mk_nop(engine, waits):
        nid[0] += 1
        nop = mybir.InstNoOp(name=f"I-wsplit-{nid[0]}", ins=[], outs=[])
        nop.engine = engine
        nop.sync_info = mybir.SyncInfo(on_wait=list(waits), on_update=[])
        return nop

    for f in nc.m.functions:
        for bb in f.blocks:
            dirty = False
            out = []
            for ins in bb.instructions:
                si = getattr(ins, "sync_info", None)
                waits = list(si.on_wait) if (si and si.on_wait) else []
                limit = mm_limit if str(ins.opcode) == "Matmult" else other_limit
                if len(waits) > limit:
                    keep = waits[-limit:] if limit > 0 else []
                    extra = waits[:-limit] if limit > 0 else waits
                    for k in range(0, len(extra), other_limit):
                        out.append(mk_nop(ins.engine, extra[k:k + other_limit]))
                    si.on_wait = keep
                    dirty = True
                out.append(ins)
            if dirty:
                bb.instructions = out


def _get_nc(mode):
    if mode not in _cache:
        _cache[mode] = _build(mode)
    return _cache[mode]


def _prep_inputs(query, key, value, w_q, w_k, w_v, w_o, mode):
    """Build per-core input maps (host-side sharding)."""
    if mode == "bf16":
        import ml_dtypes
        cast = lambda a: np.ascontiguousarray(a, dtype=ml_dtypes.bfloat16)
    else:
        cast = np.ascontiguousarray
    qT = [cast(query[b].T) for b in range(B)]
    kT = [cast(key[b].T) for b in range(B)]
    vT = [cast(value[b].T) for b in range(B)]
    in_maps = []
    for c in range(NCORES):
        b = c // 4
        h0 = (c % 4) * HPC * DK
        sl = slice(h0, h0 + HPC * DK)
        in_maps.append({
            "xq_t": qT[b],
            "xk_t": kT[b],
            "xv_t": vT[b],
            "wq_t": cast(w_q[sl, :].T),
            "wk_t": cast(w_k[sl, :].T),
            "wv_t": cast(w_v[sl, :].T),
            "wo_t": cast(w_o[:, sl].T),
        })
    return in_maps


def _assemble(results, b_o):
    out = np.empty((B, S, D), np.float32)
    for b in range(B):
        acc = results[4 * b]["out_p"].astype(np.float32)
        for c in range(4 * b + 1, 4 * b + 4):
            acc = acc + results[c]["out_p"]
        out[b] = acc + b_o[None, :]

    attn = np.empty((B, H, S, S), np.float32)

    def fix(args):
        c, h = args
        b = c // 4
        hg = (c % 4) * HPC + h
        src = results[c]["attn_t"][h]
        attn[b, hg] = src.T.astype(np.float32)

    with ThreadPoolExecutor(max_workers=16) as tp:
        list(tp.map(fix, [(c, h) for c in range(NCORES) for h in range(HPC)]))
    return out, attn


def kernel(query, key, value, w_q, w_k, w_v, w_o, b_o, _trace=False):
    from concourse.bass_utils import run_bass_kernel_spmd

    nc = _get_nc(MODE)
    in_maps = _prep_inputs(
        np.asarray(query), np.asarray(key), np.asarray(value),
        np.asarray(w_q), np.asarray(w_k), np.asarray(w_v), np.asarray(w_o),
        MODE,
    )
    res = run_bass_kernel_spmd(nc, in_maps, list(range(NCORES)), trace=_trace)
    out, attn = _assemble(res.results, np.asarray(b_o))
    if _trace:
        return (out, attn), res
    return (out, attn)


# revision 26
# speedup vs baseline: 1.1566x; 1.1566x over previous
"""MultiHeadAttention forward on 8 trn2 NeuronCores.

Sharding: 32 (batch, head) pairs across 8 cores -> 4 heads of one batch per
core (cores 0-3: batch 0, cores 4-7: batch 1).  Each core computes Q/K/V
projections for its head slice, scores softmax and attn@V, plus a partial
output projection; the host sums the 4 partials per batch (replacing the
all-reduce) and adds the bias.

On-device everything is kept feature-on-partitions ("transposed"):
  Q^T/K^T: [dk, q] ; V: [k, dv] ; scores^T strips: [k=128, q] ;
  E^T = exp(s^T/8) strips stay in SBUF; attn@V is computed as
  [V | ones]^T-augmented matmuls so softmax row sums come out of the same
  accumulation (row 64 of the PSUM tile).  Normalized attn^T is DMA'd out
  per head as [k, q]; the host transposes back to [q, k] when assembling
  the full [B, H, S, S] output.
"""

import os
import numpy as np
from concurrent.futures import ThreadPoolExecutor
from contextlib import ExitStack

os.environ.setdefault("MYCRO_LOCAL_CACHE", "1")

B, S, D, H, DK = 2, 2048, 1024, 16, 64
HPC = H // 4  # heads per core (4)
NCORES = 8
SCALE = 0.125  # 1 / sqrt(DK)

# --- configuration -------------------------------------------------------
# "f32":  E^T strips fp32, attn output fp32, q-chunk 1024 (SBUF capacity)
# "bf16": E^T strips bf16, attn output bf16 (host upcasts), q-chunk 2048
MODE = os.environ.get("MHA_MODE", "f32")
ET_BUFS = int(os.environ.get("MHA_ET_BUFS", "17"))

_cache = {}


def _build(mode):
    import concourse.bass as bass
    import concourse.mybir as mybir
    import concourse.tile as tile

    f32 = mybir.dt.float32
    f32r = mybir.dt.float32r
    bf16 = mybir.dt.bfloat16

    if mode == "bf16":
        # everything bf16 except PSUM accumulation, r, and out_p partials
        C_DT, E_DT, QCH = bf16, bf16, 1024
        ATTN_DT = f32 if os.environ.get("MHA_ATTN_F32") == "1" else bf16
        et_bufs = int(os.environ.get("MHA_ET_BUFS", "40"))
        pev_bufs = 2
    else:
        C_DT, E_DT, ATTN_DT, QCH = f32r, f32r, f32, 1024
        et_bufs = int(os.environ.get("MHA_ET_BUFS", "17"))
        pev_bufs = 1
    NQC = S // QCH

    nc = bass.Bass()

    xq_t = nc.dram_tensor("xq_t", [D, S], C_DT, kind="ExternalInput")
    xk_t = nc.dram_tensor("xk_t", [D, S], C_DT, kind="ExternalInput")
    xv_t = nc.dram_tensor("xv_t", [D, S], C_DT, kind="ExternalInput")
    wq_t = nc.dram_tensor("wq_t", [D, HPC * DK], C_DT, kind="ExternalInput")
    wk_t = nc.dram_tensor("wk_t", [D, HPC * DK], C_DT, kind="ExternalInput")
    wv_t = nc.dram_tensor("wv_t", [D, HPC * DK], C_DT, kind="ExternalInput")
    wo_t = nc.dram_tensor("wo_t", [HPC * DK, D], C_DT, kind="ExternalInput")
    attn_t = nc.dram_tensor("attn_t", [HPC, S, S], ATTN_DT, kind="ExternalOutput")
    out_p = nc.dram_tensor("out_p", [S, D], f32, kind="ExternalOutput")

    Exp = mybir.ActivationFunctionType.Exp

    with ExitStack() as ctx:
        tc = ctx.enter_context(tile.TileContext(nc))
        wpool = ctx.enter_context(tc.tile_pool(name="wpool", bufs=1))
        big = ctx.enter_context(tc.tile_pool(name="big", bufs=1))
        xstream = ctx.enter_context(tc.tile_pool(name="xstream", bufs=4))
        et_pool = ctx.enter_context(tc.tile_pool(name="et_pool", bufs=et_bufs))
        stage = ctx.enter_context(tc.tile_pool(name="stage", bufs=3))
        rpool = ctx.enter_context(tc.tile_pool(name="rpool", bufs=2))
        dpool = ctx.enter_context(tc.tile_pool(name="dpool", bufs=2, space="DRAM"))
        ps = ctx.enter_context(tc.tile_pool(name="ps", bufs=2, space="PSUM"))
        pev_pool = ctx.enter_context(tc.tile_pool(name="pev_pool", bufs=pev_bufs, space="PSUM"))

        # ---- resident tensors ----
        q_t0 = big.tile([128, S], C_DT)  # Q^T heads 0,1  [dk(2 heads), q]
        q_t1 = big.tile([128, S], C_DT)  # Q^T heads 2,3
        k_t0 = big.tile([128, S], C_DT)
        k_t1 = big.tile([128, S], C_DT)
        v_sb = big.tile([128, 16, HPC, 65], E_DT)  # [k%128, kstrip, h, dv|1]
        out_avT = big.tile([64, HPC, S], C_DT)  # attn@V result, [dv, h, q]
        wo_sb = big.tile([64, HPC, D], C_DT)  # w_o^T as [dv, h, n]

        # weights for projections — one shared slot, loaded per phase
        wq_sb = wpool.tile([128, 8, HPC * DK], C_DT, tag="w")
        wk_sb = wpool.tile([128, 8, HPC * DK], C_DT, tag="w")
        wv_sb = wpool.tile([128, 8, HPC * DK], C_DT, tag="w")

        nc.sync.dma_start(out=wq_sb, in_=wq_t[:].rearrange("(c p) d -> p c d", p=128))
        nc.sync.dma_start(out=wk_sb, in_=wk_t[:].rearrange("(c p) d -> p c d", p=128))
        nc.sync.dma_start(out=wv_sb, in_=wv_t[:].rearrange("(c p) d -> p c d", p=128))
        nc.sync.dma_start(out=wo_sb, in_=wo_t[:].rearrange("(h p) n -> p h n", p=64))
        # col 64 stays 1.0 (ones column -> softmax row sums); f32r needs a
        # plain-f32 bitcast view for memset
        nc.vector.memset(v_sb.bitcast(f32) if E_DT == f32r else v_sb, 1.0)

        # ---- phase P: projections ----
        # Q^T and K^T: [dk-group(128) x q] accumulated over 8 dm-chunks
        for name, x_dram, w_sb, dst in (
            ("q", xq_t, wq_sb, (q_t0, q_t1)),
            ("k", xk_t, wk_sb, (k_t0, k_t1)),
        ):
            for qh in range(2):  # q halves of 1024
                pg0 = ps.tile([128, 1024], f32, name=f"pg0_{name}_{qh}", tag="ps")
                pg1 = ps.tile([128, 1024], f32, name=f"pg1_{name}_{qh}", tag="ps")
                pgs = (pg0, pg1)
                for c in range(8):
                    xch = xstream.tile([128, 1024], C_DT, name=f"xch_{name}_{qh}_{c}", tag="xs")
                    nc.sync.dma_start(
                        out=xch,
                        in_=x_dram[c * 128:(c + 1) * 128, qh * 1024:(qh + 1) * 1024],
                    )
                    for g in range(2):
                        for jj in range(2):
                            nc.tensor.matmul(
                                pgs[g][:, jj * 512:(jj + 1) * 512],
                                w_sb[:, c, g * 128:(g + 1) * 128],
                                xch[:, jj * 512:(jj + 1) * 512],
                                start=(c == 0),
                                stop=(c == 7),
                            )
                for g in range(2):
                    nc.scalar.copy(
                        out=dst[g][:, qh * 1024:(qh + 1) * 1024], in_=pgs[g]
                    )

        # V: [k x dv] accumulated over 8 dm-chunks, per k-strip
        for i in range(16):
            xvc = xstream.tile([128, 8, 128], C_DT, name=f"xvc_{i}", tag="xs")
            nc.sync.dma_start(
                out=xvc,
                in_=xv_t[:].rearrange("(c p) s -> p c s", p=128)[:, :, i * 128:(i + 1) * 128],
            )
            pv = ps.tile([128, HPC * DK], f32, name=f"pv_{i}", tag="ps")
            for c in range(8):
                nc.tensor.matmul(
                    pv,
                    xvc[:, c, :],
                    wv_sb[:, c, :],
                    start=(c == 0),
                    stop=(c == 7),
                )
            nc.scalar.copy(out=v_sb[:, i, :, 0:64], in_=pv[:].rearrange("p (h d) -> p h d", h=HPC))

        # ---- phase A: attention, q-chunk outer so the output projection
        # for chunk qc overlaps the attention of chunk qc+1 ----
        for qc in range(NQC):
            for h in range(HPC):
                q_t = (q_t0, q_t1)[h // 2]
                k_t = (k_t0, k_t1)[h // 2]
                off = (h % 2) * 64
                q0 = qc * QCH
                ets = []
                for i in range(16):
                    et = et_pool.tile([128, QCH], E_DT, name=f"et_{h}_{qc}_{i}", tag="et")
                    ets.append(et)
                    for qq in range(QCH // 1024):
                        pss = ps.tile([128, 1024], f32, name=f"pss_{h}_{qc}_{i}_{qq}", tag="ps")
                        for j2 in range(2):
                            qlo = q0 + qq * 1024 + j2 * 512
                            nc.tensor.matmul(
                                pss[:, j2 * 512:(j2 + 1) * 512],
                                k_t[off:off + 64, i * 128:(i + 1) * 128],
                                q_t[off:off + 64, qlo:qlo + 512],
                                start=True,
                                stop=True,
                            )
                        nc.scalar.activation(
                            out=et[:, qq * 1024:(qq + 1) * 1024],
                            in_=pss,
                            func=Exp,
                            scale=SCALE,
                        )
                # attn @ V with ones-augmented V: row 64 = softmax sums.
                # i-outer: EV matmuls for strip i become ready as soon as
                # exp(i) lands, so the PE can fill exp-wait gaps.
                pev = pev_pool.tile([65, QCH], f32, name=f"pev_{h}_{qc}", tag="pev")
                for i in range(16):
                    for j in range(QCH // 512):
                        nc.tensor.matmul(
                            pev[:, j * 512:(j + 1) * 512],
                            v_sb[:, i, h, :],
                            ets[i][:, j * 512:(j + 1) * 512],
                            start=(i == 0),
                            stop=(i == 15),
                        )
                # Softmax denominators: DVE reciprocal costs ~6.5ns per
                # FREE element regardless of partition count, so bounce the
                # [1,QCH] rowsums through DRAM into a [128, QCH/128] layout,
                # reciprocal there (free dim = QCH/128), bounce back, and
                # broadcast-load to all partitions.
                r_sb = rpool.tile([1, QCH], f32, name=f"r_sb_{h}_{qc}", tag="r", bufs=2)
                nc.vector.tensor_copy(out=r_sb, in_=pev[64:65, :])
                r_dram = dpool.tile([1, QCH], f32, name=f"r_dram_{h}_{qc}", tag="rd")
                nc.sync.dma_start(out=r_dram, in_=r_sb)
                rs_sm = rpool.tile([128, QCH // 128], f32, name=f"rs_sm_{h}_{qc}", tag="rs", bufs=2)
                nc.sync.dma_start(
                    out=rs_sm, in_=r_dram[0].rearrange("(p c) -> p c", p=128))
                nc.vector.reciprocal(out=rs_sm, in_=rs_sm)
                r2_dram = dpool.tile([128, QCH // 128], f32, name=f"r2_dram_{h}_{qc}", tag="rd2")
                nc.sync.dma_start(out=r2_dram, in_=rs_sm)
                rb = rpool.tile([128, QCH], f32 if E_DT == f32r else E_DT,
                                name=f"rb_{h}_{qc}", tag="rb")
                rb_src = r2_dram[:].rearrange("p c -> (p c)")[None, :].to_broadcast([128, QCH])
                if rb.dtype == f32:
                    nc.sync.dma_start(out=rb, in_=rb_src)
                else:
                    nc.gpsimd.dma_start(out=rb, in_=rb_src)  # casting DMA
                # normalized attn@V -> out_avT
                nc.vector.tensor_mul(
                    out_avT[:, h, q0:q0 + QCH], pev[0:64, :], rb[0:64, :]
                )
                # normalized attn strips -> HBM (transposed layout [k, q])
                for i in range(16):
                    ast = stage.tile([128, QCH], ATTN_DT, name=f"ast_{h}_{qc}_{i}", tag="ast")
                    nc.vector.tensor_mul(ast, ets[i], rb)
                    nc.sync.dma_start(
                        out=attn_t[h, i * 128:(i + 1) * 128, q0:q0 + QCH], in_=ast
                    )

            # ---- output projection partial for this q-chunk ----
            for qs in range(qc * (QCH // 128), (qc + 1) * (QCH // 128)):
                po = ps.tile([128, D], f32, name=f"po_{qs}", tag="ps")
                for n2 in range(2):
                    for h in range(HPC):
                        nc.tensor.matmul(
                            po[:, n2 * 512:(n2 + 1) * 512],
                            out_avT[:, h, qs * 128:(qs + 1) * 128],
                            wo_sb[:, h, n2 * 512:(n2 + 1) * 512],
                            start=(h == 0),
                            stop=(h == 3),
                        )
                ost = stage.tile([128, D], f32, name=f"ost_{qs}", tag="ast")
                nc.vector.tensor_copy(out=ost, in_=po)
                nc.sync.dma_start(out=out_p[qs * 128:(qs + 1) * 128, :], in_=ost)

    _split_waits(nc, mybir)
    return nc


def _split_waits(nc, mybir, mm_limit=1, other_limit=1):
    """The walrus build in this env accepts only a small number of sync-wait
    commands per instruction (matmul LDWEIGHTS appears to take just one).
    Hoist excess waits onto injected same-engine NoOps, which execute the
    waits in order before the real instruction."""
    nid = [0]

    def mk_nop(engine, waits):
        nid[0] += 1
        nop = mybir.InstNoOp(name=f"I-wsplit-{nid[0]}", ins=[], outs=[])
        nop.engine = engine
        nop.sync_info = mybir.SyncInfo(on_wait=list(waits), on_update=[])
        return nop

    for f in nc.m.functions:
        for bb in f.blocks:
            dirty = False
            out = []
            for ins in bb.instructions:
                si = getattr(ins, "sync_info", None)
                waits = list(si.on_wait) if (si and si.on_wait) else []
                limit = mm_limit if str(ins.opcode) == "Matmult" else other_limit
                if len(waits) > limit:
                    keep = waits[-limit:] if limit > 0 else []
                    extra = waits[:-limit] if limit > 0 else waits
                    for k in range(0, len(extra), other_limit):
                        out.append(mk_nop(ins.engine, extra[k:k + other_limit]))
                    si.on_wait = keep
                    dirty = True
                out.append(ins)
            if dirty:
                bb.instructions = out


def _get_nc(mode):
    if mode not in _cache:
        _cache[mode] = _build(mode)
    return _cache[mode]


def _prep_inputs(query, key, value, w_q, w_k, w_v, w_o, mode):
    """Build per-core input maps (host-side sharding)."""
    if mode == "bf16":
        import ml_dtypes
        cast = lambda a: np.ascontiguousarray(a, dtype=ml_dtypes.bfloat16)
    else:
        cast = np.ascontiguousarray
    qT = [cast(query[b].T) for b in range(B)]
    kT = [cast(key[b].T) for b in range(B)]
    vT = [cast(value[b].T) for b in range(B)]
    in_maps = []
    for c in range(NCORES):
        b = c // 4
        h0 = (c % 4) * HPC * DK
        sl = slice(h0, h0 + HPC * DK)
        in_maps.append({
            "xq_t": qT[b],
            "xk_t": kT[b],
            "xv_t": vT[b],
            "wq_t": cast(w_q[sl, :].T),
            "wk_t": cast(w_k[sl, :].T),
            "wv_t": cast(w_v[sl, :].T),
            "wo_t": cast(w_o[:, sl].T),
        })
    return in_maps


def _assemble(results, b_o):
    out = np.empty((B, S, D), np.float32)
    for b in range(B):
        acc = results[4 * b]["out_p"].astype(np.float32)
        for c in range(4 * b + 1, 4 * b + 4):
            acc = acc + results[c]["out_p"]
        out[b] = acc + b_o[None, :]

    attn = np.empty((B, H, S, S), np.float32)

    def fix(args):
        c, h = args
        b = c // 4
        hg = (c % 4) * HPC + h
        src = results[c]["attn_t"][h]
        attn[b, hg] = src.T.astype(np.float32)

    with ThreadPoolExecutor(max_workers=16) as tp:
        list(tp.map(fix, [(c, h) for c in range(NCORES) for h in range(HPC)]))
    return out, attn


def kernel(query, key, value, w_q, w_k, w_v, w_o, b_o, _trace=False):
    from concourse.bass_utils import run_bass_kernel_spmd

    nc = _get_nc(MODE)
    in_maps = _prep_inputs(
        np.asarray(query), np.asarray(key), np.asarray(value),
        np.asarray(w_q), np.asarray(w_k), np.asarray(w_v), np.asarray(w_o),
        MODE,
    )
    res = run_bass_kernel_spmd(nc, in_maps, list(range(NCORES)), trace=_trace)
    out, attn = _assemble(res.results, np.asarray(b_o))
    if _trace:
        return (out, attn), res
    return (out, attn)
